# revision 45
# baseline (speedup 1.0000x reference)
"""Trainium2 Bass kernel for nn_AffinityBiFC.

Reference computation (B=4, N=M=128, D=256, BD=1024):
    t  = einsum('bnd,dek->bnek', X, A)
    bi = einsum('bnek,bme->bnmk', t, Y)
    S  = einsum('bnmk,ok->bnmo', bi, W) + b        -> S[..., 0]  [B, N, M]

Algebraic collapse (exact reassociation):
    Aw[d, e] = sum_k A[d, e, k] * W[0, k]          # one streaming pass over A (268 MB)
    S[b]     = X[b] @ Aw @ Y[b].T + b              # tiny matmuls

Sharding: A is split over its first (d) axis across the 8 cores (each core
streams a contiguous 33.5 MB block and produces 32 rows of Aw).  Each core
computes its partial S (full [B, N, M] shape, contracting only its own 32
d-rows); a final fp16 ReduceScatter(add) over the n axis leaves each core
with its 16-row slice of the summed S, and the host concatenates the slices
(and adds the bias).

Per-core pipeline (HW-measured 128-155 us depending on collective peer
skew; the A stream runs at the ~322 GB/s per-core HBM ceiling ~= 104 us and
the ReduceScatter triggers ~118-125 us in):
  - One DMA per d-row ([128, 2, 1024] tiles, 1 MB each, all on the sync
    queue) so compute trails the stream by at most one row.  Partition p
    holds the e-pair (2p, 2p+1): each partition's row is ONE 8 KB
    contiguous DRAM run, which quarters the DMA_DIRECT2D issue cost vs the
    4 KB (e%128) layout and keeps the sync queue far ahead of the stream.
    W rides the scalar queue in parallel; only small timing-noncritical
    inputs use the (several-times slower) gpsimd queue.  The last row is
    split per-parity to shorten the end-of-stream drain.
  - DVE tensor_tensor (A_tile(fp32) * W_rep(fp16) -> fp32 prod); ACT
    activation(Copy, accum_out) sums over k -> acc[e_pair, par, dl].  6 of
    the 64 reduces run on DVE tensor_reduce instead, balancing DVE/ACT
    (~85 us each) so both stay under the stream even when DVFS throttles
    a core.  W is fp16 so it lands faster alongside row 0.
  - Per transpose group (8,8,8,7,1 rows): PE-transpose the acc slice,
    cast fp16, and fold into T^T[e, bn] += Aw_g^T @ X^T in PSUM — all
    hidden under the stream; only the 1-row group runs after it.
  - Tail: eight 128x128x128 fp16 S matmuls (S[n,m] += sum_par T^T.T Y^T
    per b), DMA partial S (fp16) to DRAM, ReduceScatter, DMA the 16-row
    slice to out.  A tiny warm-up AllReduce at kernel start absorbs the
    ~50 us ncfw cold-start so the real collective triggers in ~1 us.
"""

import numpy as np

B, N, D, KD = 4, 128, 256, 1024
P = 128
C = 8                   # cores
DL = D // C             # 32 d-rows per core

_cached = {}


def _build_program():
    import concourse.bass as bass
    import concourse.mybir as mybir
    import concourse.tile as tile
    from concourse import bacc
    from concourse.masks import make_identity

    fp32 = mybir.dt.float32
    fp16 = mybir.dt.float16

    nc = bacc.Bacc(
        "TRN2",
        target_bir_lowering=False,
        debug=False,
        num_devices=C,
    )

    a_sh = nc.dram_tensor("a_sh", [DL, D, KD], fp32, kind="ExternalInput").ap()
    # host-staged per-core: X^T fp16 rows d in [32c, 32c+32), layout [d, b*n]
    xt_in = nc.dram_tensor("xt_in", [DL, B * N], fp16, kind="ExternalInput").ap()
    # host-staged full: Y^T fp16 [e_pair, par, b, m] (e = 2*e_pair + par)
    yt_in = nc.dram_tensor("yt_in", [P, 2, B, N], fp16, kind="ExternalInput").ap()
    w_rep = nc.dram_tensor("w_rep", [P, KD], fp16, kind="ExternalInput").ap()
    # per-core output: n-rows [16c, 16c+16) of S in [n, b, m] layout (fp16;
    # the host casts to fp32 when assembling)
    out = nc.dram_tensor("out", [N // C, B * N], fp16, kind="ExternalOutput").ap()

    with tile.TileContext(nc) as tc:
        with (
            tc.tile_pool(name="apool", bufs=8) as apool,
            tc.tile_pool(name="ppool", bufs=4) as ppool,
            tc.tile_pool(name="sbuf", bufs=1) as sbuf,
            tc.tile_pool(name="pstr", bufs=2, space="PSUM") as pstr,
            tc.tile_pool(name="psT", bufs=2, space="PSUM") as psTp,
            tc.tile_pool(name="psS", bufs=2, space="PSUM") as psSp,
            tc.tile_pool(name="dram", bufs=1, space="DRAM") as dram,
        ):
            # W on the scalar queue (a hardware DGE queue like sync's) so it
            # lands in parallel with row 0 on sync; gpsimd DMAs are several
            # times slower, so only small timing-noncritical inputs ride there.
            a_flat = a_sh.rearrange("dl (p two) k -> p dl two k", two=2)
            w_sb = sbuf.tile([P, KD], fp16)
            nc.scalar.dma_start(w_sb[:], w_rep[:])
            # transpose-group sizes: big groups hidden under the stream, a
            # 1-row final group so the post-stream chain is minimal
            TGROUPS = [8, 8, 8, 7, 1]
            assert sum(TGROUPS) == DL
            TG8 = len(TGROUPS)
            gends = []
            for gn in TGROUPS:
                gends.append((gends[-1] if gends else 0) + gn)
            xt_g = []
            g0 = 0
            for g, gn in enumerate(TGROUPS):
                t = sbuf.tile([gn, B * N], fp16, name=f"xt{g}")
                nc.gpsimd.dma_start(t[:], xt_in[g0 : g0 + gn])
                xt_g.append(t)
                g0 += gn
            yT = sbuf.tile([P, 2, B, N], fp16)  # [e_pair, par, b, m]
            nc.gpsimd.dma_start(yT[:], yt_in[:])

            # warm-up collective: absorbs the ncfw cold-start so the real
            # AllReduce runs with ~1us trigger latency.
            warm_in = dram.tile([1, 16], fp32)
            warm_out = dram.tile([1, 16], fp32, addr_space="Shared")
            nc.gpsimd.dma_start(warm_in[:], w_rep[0:1, 0:16])
            nc.gpsimd.collective_compute(
                "AllReduce",
                mybir.AluOpType.add,
                replica_groups=[list(range(C))],
                ins=[warm_in.opt()],
                outs=[warm_out.opt()],
            )

            # acc[e_pair, par, dl] = Aw[c*DL + dl, 2*e_pair + par]
            acc = sbuf.tile([P, 2, DL], fp32)
            scratch = sbuf.tile([P, KD], fp32)

            ident = sbuf.tile([P, P], fp32)
            make_identity(nc, ident)

            # T^T[e, bn] accumulates in PSUM over transpose groups during the
            # stream (PE is otherwise idle); psT[par][e_pair, bn]
            psT = [
                psTp.tile([P, B * N], fp32, name=f"psT{par}", tag=f"T{par}", bufs=1)
                for par in range(2)
            ]

            # main stream: one DMA per d-row (partition = e-pair -> 8 KB runs);
            # last row split per-parity to cut the end-of-stream drain
            half = sbuf.tile([P, 1], fp32)  # k-half partial for the last tile
            for r in range(DL):
                at = apool.tile([P, 2, KD], fp32, tag="a", name=f"at{r}")
                last = r == DL - 1
                if last:
                    # final row: per-parity DMAs, and the very last 512 KB
                    # piece split in k-halves so its reduce runs on ACT and
                    # DVE in parallel right as the stream ends
                    nc.sync.dma_start(at[:, 0, :], a_flat[:, r, 0, :])
                    nc.sync.dma_start(at[:, 1, : KD // 2], a_flat[:, r, 1, : KD // 2])
                    nc.sync.dma_start(at[:, 1, KD // 2 :], a_flat[:, r, 1, KD // 2 :])
                else:
                    nc.sync.dma_start(at[:], a_flat[:, r, :, :])
                for par in range(2):
                    prod = ppool.tile([P, KD], fp32, tag="prod", name=f"pr{r}{par}")
                    if last and par == 1:
                        kh = KD // 2
                        nc.vector.tensor_tensor(
                            out=prod[:, :kh],
                            in0=at[:, 1, :kh],
                            in1=w_sb[:, :kh],
                            op=mybir.AluOpType.mult,
                        )
                        nc.scalar.activation(
                            out=scratch[:, :kh],
                            in_=prod[:, :kh],
                            func=mybir.ActivationFunctionType.Copy,
                            accum_out=half[:],
                        )
                        nc.vector.tensor_tensor(
                            out=prod[:, kh:],
                            in0=at[:, 1, kh:],
                            in1=w_sb[:, kh:],
                            op=mybir.AluOpType.mult,
                        )
                        nc.vector.tensor_reduce(
                            out=acc[:, 1, r : r + 1],
                            in_=prod[:, kh:],
                            axis=mybir.AxisListType.X,
                            op=mybir.AluOpType.add,
                        )
                        nc.vector.tensor_tensor(
                            out=acc[:, 1, r : r + 1],
                            in0=acc[:, 1, r : r + 1],
                            in1=half[:],
                            op=mybir.AluOpType.add,
                        )
                        continue
                    nc.vector.tensor_tensor(
                        out=prod[:],
                        in0=at[:, par, :],
                        in1=w_sb,
                        op=mybir.AluOpType.mult,
                    )
                    if par == 0 and r % 4 == 2 and r < 24:
                        # rebalance: ACT (64x1.43us) is the longest engine
                        # chain; push 6 of the k-reduces onto DVE
                        nc.vector.tensor_reduce(
                            out=acc[:, par, r : r + 1],
                            in_=prod[:],
                            axis=mybir.AxisListType.X,
                            op=mybir.AluOpType.add,
                        )
                    else:
                        nc.scalar.activation(
                            out=scratch[:],
                            in_=prod[:],
                            func=mybir.ActivationFunctionType.Copy,
                            accum_out=acc[:, par, r : r + 1],
                        )
                if r + 1 in gends:
                    # fold this group of Aw rows into T^T while streaming:
                    # transpose -> awt[gn, e_pair] fp16 -> psT[par] += awt.T @ X^T
                    g = gends.index(r + 1)
                    gn = TGROUPS[g]
                    glo = r + 1 - gn
                    for par in range(2):
                        pst = pstr.tile([gn, P], fp32, tag="tr", name=f"tr{g}{par}")
                        nc.tensor.transpose(pst[:], acc[:, par, glo : r + 1], ident)
                        awt = ppool.tile([gn, P], fp16, tag="awt", name=f"awt{g}{par}")
                        nc.scalar.copy(out=awt[:], in_=pst[:])
                        nc.tensor.matmul(
                            psT[par],
                            lhsT=awt[:],
                            rhs=xt_g[g][:],
                            start=(g == 0),
                            stop=(g == TG8 - 1),
                        )

            # tail: copy T^T to SBUF, then S matmuls.  par 1 lands last, so
            # its copy is split across vector+scalar to halve the chain.
            tT = sbuf.tile([P, 2, B, N], fp16)  # [e_pair, par, b, n]
            nc.vector.tensor_copy(out=tT[:, 0, :, :], in_=psT[0])
            nc.vector.tensor_copy(out=tT[:, 1, 0:2, :], in_=psT[1][:, 0 : 2 * N])
            nc.scalar.copy(out=tT[:, 1, 2:4, :], in_=psT[1][:, 2 * N : 4 * N])

            # S[b][n, m] = sum_par T^T[:, par, b, :].T @ Y^T[:, par, b, :]
            s_sb = sbuf.tile([P, B, N], fp16)  # [n, b, m]
            for b in range(B):
                psS = psSp.tile([P, N], fp32, tag="S", name=f"psS{b}")
                for par in range(2):
                    nc.tensor.matmul(
                        psS,
                        lhsT=tT[:, par, b, :],
                        rhs=yT[:, par, b, :],
                        start=(par == 0),
                        stop=(par == 1),
                    )
                nc.scalar.copy(out=s_sb[:, b, :], in_=psS)

            # partial S -> DRAM -> ReduceScatter(add, fp16) over n.  One DMA:
            # per-b DMAs serialize ~0.7us each on the sync queue and delay
            # the collective trigger.
            cc_in = dram.tile([P, B * N], fp16)
            cc_out = dram.tile([N // C, B * N], fp16)
            nc.sync.dma_start(cc_in[:], s_sb[:])
            nc.gpsimd.collective_compute(
                "ReduceScatter",
                mybir.AluOpType.add,
                replica_groups=[list(range(C))],
                ins=[cc_in.opt()],
                outs=[cc_out.opt()],
            )
            nc.sync.dma_start(out[:], cc_out[:])

    nc.compile()
    return nc


def _get_program():
    if "nc" not in _cached:
        _cached["nc"] = _build_program()
    return _cached["nc"]


def _run(X, Y, A, W, b, trace=False, **trace_kwargs):
    from concourse.bass_utils import run_bass_kernel_spmd

    nc = _get_program()

    A = np.ascontiguousarray(A, dtype=np.float32)
    W = np.ascontiguousarray(W, dtype=np.float32)
    X = np.asarray(X, dtype=np.float32)
    Y = np.asarray(Y, dtype=np.float32)
    xt = np.ascontiguousarray(
        X.transpose(2, 0, 1).reshape(D, B * N), dtype=np.float16
    )  # [d, b*n]
    # [e_pair, par, b, m]: e = 2*e_pair + par
    yt = np.ascontiguousarray(
        Y.transpose(2, 0, 1).reshape(P, 2, B, N), dtype=np.float16
    )
    w_rep = np.ascontiguousarray(
        np.broadcast_to(W.reshape(1, KD), (P, KD)), dtype=np.float16
    )

    core_ids = list(range(C))
    in_maps = [
        {
            "a_sh": A[c * DL : (c + 1) * DL],
            "xt_in": np.ascontiguousarray(xt[c * DL : (c + 1) * DL]),
            "yt_in": yt,
            "w_rep": w_rep,
        }
        for c in core_ids
    ]

    res = run_bass_kernel_spmd(nc, in_maps, core_ids, trace=trace, **trace_kwargs)
    # each core returns its ReduceScatter n-slice [16, B*N]; concat + relayout
    s_nbm = np.concatenate(
        [np.asarray(res.results[c]["out"], dtype=np.float32) for c in core_ids],
        axis=0,
    ).reshape(N, B, N)
    out = np.ascontiguousarray(s_nbm.transpose(1, 0, 2))
    out = out + np.float32(b.reshape(-1)[0])
    return out, res


def kernel(X, Y, A, W, b):
    out, _ = _run(X, Y, A, W, b, trace=False)
    return out


# revision 46
# speedup vs baseline: 1.0475x; 1.0475x over previous
"""Trainium2 Bass kernel for nn_AffinityBiFC.

Reference computation (B=4, N=M=128, D=256, BD=1024):
    t  = einsum('bnd,dek->bnek', X, A)
    bi = einsum('bnek,bme->bnmk', t, Y)
    S  = einsum('bnmk,ok->bnmo', bi, W) + b        -> S[..., 0]  [B, N, M]

Algebraic collapse (exact reassociation):
    Aw[d, e] = sum_k A[d, e, k] * W[0, k]          # one streaming pass over A (268 MB)
    S[b]     = X[b] @ Aw @ Y[b].T + b              # tiny matmuls

Sharding: A is split over its first (d) axis across the 8 cores (each core
streams a contiguous 33.5 MB block and produces 32 rows of Aw).  Each core
computes its partial S (full [B, N, M] shape, contracting only its own 32
d-rows); a final fp16 ReduceScatter(add) over the n axis leaves each core
with its 16-row slice of the summed S, and the host concatenates the slices
(and adds the bias).

Per-core pipeline (HW-measured 128-155 us depending on collective peer
skew; the A stream runs at the ~322 GB/s per-core HBM ceiling ~= 104 us and
the ReduceScatter triggers ~118-125 us in):
  - One DMA per d-row ([128, 2, 1024] tiles, 1 MB each, all on the sync
    queue) so compute trails the stream by at most one row.  Partition p
    holds the e-pair (2p, 2p+1): each partition's row is ONE 8 KB
    contiguous DRAM run, which quarters the DMA_DIRECT2D issue cost vs the
    4 KB (e%128) layout and keeps the sync queue far ahead of the stream.
    W rides the scalar queue in parallel; only small timing-noncritical
    inputs use the (several-times slower) gpsimd queue.  The last row is
    split per-parity to shorten the end-of-stream drain.
  - DVE tensor_tensor (A_tile(fp32) * W_rep(fp16) -> fp32 prod); ACT
    activation(Copy, accum_out) sums over k -> acc[e_pair, par, dl].  6 of
    the 64 reduces run on DVE tensor_reduce instead, balancing DVE/ACT
    (~85 us each) so both stay under the stream even when DVFS throttles
    a core.  W is fp16 so it lands faster alongside row 0.
  - Per transpose group (8,8,8,7,1 rows): PE-transpose the acc slice,
    cast fp16, and fold into T^T[e, bn] += Aw_g^T @ X^T in PSUM — all
    hidden under the stream; only the 1-row group runs after it.
  - Tail: eight 128x128x128 fp16 S matmuls (S[n,m] += sum_par T^T.T Y^T
    per b), DMA partial S (fp16) to DRAM, ReduceScatter, DMA the 16-row
    slice to out.  A tiny warm-up AllReduce at kernel start absorbs the
    ~50 us ncfw cold-start so the real collective triggers in ~1 us.
"""

import numpy as np

B, N, D, KD = 4, 128, 256, 1024
P = 128
C = 8                   # cores
DL = D // C             # 32 d-rows per core

_cached = {}


def _build_program():
    import concourse.bass as bass
    import concourse.mybir as mybir
    import concourse.tile as tile
    from concourse import bacc
    from concourse.masks import make_identity

    fp32 = mybir.dt.float32
    fp16 = mybir.dt.float16

    nc = bacc.Bacc(
        "TRN2",
        target_bir_lowering=False,
        debug=False,
        num_devices=C,
    )

    a_sh = nc.dram_tensor("a_sh", [DL, D, KD], fp32, kind="ExternalInput").ap()
    # host-staged per-core: X^T fp16 rows d in [32c, 32c+32), layout [d, b*n]
    xt_in = nc.dram_tensor("xt_in", [DL, B * N], fp16, kind="ExternalInput").ap()
    # host-staged full: Y^T fp16 [e_pair, par, b, m] (e = 2*e_pair + par)
    yt_in = nc.dram_tensor("yt_in", [P, 2, B, N], fp16, kind="ExternalInput").ap()
    w_rep = nc.dram_tensor("w_rep", [P, KD], fp16, kind="ExternalInput").ap()
    # per-core output: n-rows [16c, 16c+16) of S in [n, b, m] layout (fp16;
    # the host casts to fp32 when assembling)
    out = nc.dram_tensor("out", [N // C, B * N], fp16, kind="ExternalOutput").ap()

    with tile.TileContext(nc) as tc:
        with (
            tc.tile_pool(name="apool", bufs=8) as apool,
            tc.tile_pool(name="ppool", bufs=6) as ppool,
            tc.tile_pool(name="sbuf", bufs=1) as sbuf,
            tc.tile_pool(name="pstr", bufs=2, space="PSUM") as pstr,
            tc.tile_pool(name="psT", bufs=2, space="PSUM") as psTp,
            tc.tile_pool(name="psS", bufs=2, space="PSUM") as psSp,
            tc.tile_pool(name="dram", bufs=1, space="DRAM") as dram,
        ):
            # W on the scalar queue (a hardware DGE queue like sync's) so it
            # lands in parallel with row 0 on sync; gpsimd DMAs are several
            # times slower, so only small timing-noncritical inputs ride there.
            a_flat = a_sh.rearrange("dl (p two) k -> p dl two k", two=2)
            w_sb = sbuf.tile([P, KD], fp16)
            nc.scalar.dma_start(w_sb[:], w_rep[:])
            # transpose-group sizes: big groups hidden under the stream, a
            # 1-row final group so the post-stream chain is minimal
            TGROUPS = [8, 8, 8, 7, 1]
            assert sum(TGROUPS) == DL
            TG8 = len(TGROUPS)
            gends = []
            for gn in TGROUPS:
                gends.append((gends[-1] if gends else 0) + gn)
            xt_g = []
            g0 = 0
            for g, gn in enumerate(TGROUPS):
                t = sbuf.tile([gn, B * N], fp16, name=f"xt{g}")
                nc.gpsimd.dma_start(t[:], xt_in[g0 : g0 + gn])
                xt_g.append(t)
                g0 += gn
            yT = sbuf.tile([P, 2, B, N], fp16)  # [e_pair, par, b, m]
            nc.gpsimd.dma_start(yT[:], yt_in[:])

            # warm-up collective: absorbs the ncfw cold-start so the real
            # AllReduce runs with ~1us trigger latency.
            warm_in = dram.tile([1, 16], fp32)
            warm_out = dram.tile([1, 16], fp32, addr_space="Shared")
            nc.gpsimd.dma_start(warm_in[:], w_rep[0:1, 0:16])
            nc.gpsimd.collective_compute(
                "AllReduce",
                mybir.AluOpType.add,
                replica_groups=[list(range(C))],
                ins=[warm_in.opt()],
                outs=[warm_out.opt()],
            )

            # acc[e_pair, par, dl] = Aw[c*DL + dl, 2*e_pair + par]
            acc = sbuf.tile([P, 2, DL], fp32)
            scratch = sbuf.tile([P, KD], fp32)

            ident = sbuf.tile([P, P], fp32)
            make_identity(nc, ident)

            # T^T[e, bn] accumulates in PSUM over transpose groups during the
            # stream (PE is otherwise idle); psT[par][e_pair, bn]
            psT = [
                psTp.tile([P, B * N], fp32, name=f"psT{par}", tag=f"T{par}", bufs=1)
                for par in range(2)
            ]

            # main stream: one DMA per d-row (partition = e-pair -> 8 KB runs);
            # last row split per-parity to cut the end-of-stream drain
            half = sbuf.tile([P, 1], fp32)  # k-half partial for the last tile
            for r in range(DL):
                at = apool.tile([P, 2, KD], fp32, tag="a", name=f"at{r}")
                last = r == DL - 1
                if last:
                    # final row: per-parity DMAs, and the very last 512 KB
                    # piece split in k-halves so its reduce runs on ACT and
                    # DVE in parallel right as the stream ends
                    nc.sync.dma_start(at[:, 0, :], a_flat[:, r, 0, :])
                    nc.sync.dma_start(at[:, 1, : KD // 2], a_flat[:, r, 1, : KD // 2])
                    nc.sync.dma_start(at[:, 1, KD // 2 :], a_flat[:, r, 1, KD // 2 :])
                else:
                    nc.sync.dma_start(at[:], a_flat[:, r, :, :])
                for par in range(2):
                    prod = ppool.tile([P, KD], fp32, tag="prod", name=f"pr{r}{par}")
                    if last and par == 1:
                        kh = KD // 2
                        nc.vector.tensor_tensor(
                            out=prod[:, :kh],
                            in0=at[:, 1, :kh],
                            in1=w_sb[:, :kh],
                            op=mybir.AluOpType.mult,
                        )
                        nc.scalar.activation(
                            out=scratch[:, :kh],
                            in_=prod[:, :kh],
                            func=mybir.ActivationFunctionType.Copy,
                            accum_out=half[:],
                        )
                        nc.vector.tensor_tensor(
                            out=prod[:, kh:],
                            in0=at[:, 1, kh:],
                            in1=w_sb[:, kh:],
                            op=mybir.AluOpType.mult,
                        )
                        nc.vector.tensor_reduce(
                            out=acc[:, 1, r : r + 1],
                            in_=prod[:, kh:],
                            axis=mybir.AxisListType.X,
                            op=mybir.AluOpType.add,
                        )
                        nc.vector.tensor_tensor(
                            out=acc[:, 1, r : r + 1],
                            in0=acc[:, 1, r : r + 1],
                            in1=half[:],
                            op=mybir.AluOpType.add,
                        )
                        continue
                    nc.vector.tensor_tensor(
                        out=prod[:],
                        in0=at[:, par, :],
                        in1=w_sb,
                        op=mybir.AluOpType.mult,
                    )
                    if par == 0 and r % 4 == 2 and r < 24:
                        # rebalance: ACT (64x1.43us) is the longest engine
                        # chain; push 6 of the k-reduces onto DVE
                        nc.vector.tensor_reduce(
                            out=acc[:, par, r : r + 1],
                            in_=prod[:],
                            axis=mybir.AxisListType.X,
                            op=mybir.AluOpType.add,
                        )
                    else:
                        nc.scalar.activation(
                            out=scratch[:],
                            in_=prod[:],
                            func=mybir.ActivationFunctionType.Copy,
                            accum_out=acc[:, par, r : r + 1],
                        )
                if r + 1 in gends:
                    # fold this group of Aw rows into T^T while streaming:
                    # transpose -> awt[gn, e_pair] fp16 -> psT[par] += awt.T @ X^T
                    g = gends.index(r + 1)
                    gn = TGROUPS[g]
                    glo = r + 1 - gn
                    for par in range(2):
                        pst = pstr.tile([gn, P], fp32, tag="tr", name=f"tr{g}{par}")
                        nc.tensor.transpose(pst[:], acc[:, par, glo : r + 1], ident)
                        awt = ppool.tile([gn, P], fp16, tag="awt", name=f"awt{g}{par}")
                        nc.scalar.copy(out=awt[:], in_=pst[:])
                        nc.tensor.matmul(
                            psT[par],
                            lhsT=awt[:],
                            rhs=xt_g[g][:],
                            start=(g == 0),
                            stop=(g == TG8 - 1),
                        )

            # tail: copy T^T to SBUF, then S matmuls.  par 1 lands last, so
            # its copy is split across vector+scalar to halve the chain.
            tT = sbuf.tile([P, 2, B, N], fp16)  # [e_pair, par, b, n]
            nc.vector.tensor_copy(out=tT[:, 0, :, :], in_=psT[0])
            nc.vector.tensor_copy(out=tT[:, 1, 0:2, :], in_=psT[1][:, 0 : 2 * N])
            nc.scalar.copy(out=tT[:, 1, 2:4, :], in_=psT[1][:, 2 * N : 4 * N])

            # S[b][n, m] = sum_par T^T[:, par, b, :].T @ Y^T[:, par, b, :]
            s_sb = sbuf.tile([P, B, N], fp16)  # [n, b, m]
            for b in range(B):
                psS = psSp.tile([P, N], fp32, tag="S", name=f"psS{b}")
                for par in range(2):
                    nc.tensor.matmul(
                        psS,
                        lhsT=tT[:, par, b, :],
                        rhs=yT[:, par, b, :],
                        start=(par == 0),
                        stop=(par == 1),
                    )
                if b % 2 == 0:
                    nc.vector.tensor_copy(out=s_sb[:, b, :], in_=psS)
                else:
                    nc.scalar.copy(out=s_sb[:, b, :], in_=psS)

            # partial S -> DRAM -> ReduceScatter(add, fp16) over n.  One DMA:
            # per-b DMAs serialize ~0.7us each on the sync queue and delay
            # the collective trigger.
            cc_in = dram.tile([P, B * N], fp16)
            cc_out = dram.tile([N // C, B * N], fp16)
            nc.sync.dma_start(cc_in[:], s_sb[:])
            nc.gpsimd.collective_compute(
                "ReduceScatter",
                mybir.AluOpType.add,
                replica_groups=[list(range(C))],
                ins=[cc_in.opt()],
                outs=[cc_out.opt()],
            )
            nc.sync.dma_start(out[:], cc_out[:])

    nc.compile()
    return nc


def _get_program():
    if "nc" not in _cached:
        _cached["nc"] = _build_program()
    return _cached["nc"]


def _run(X, Y, A, W, b, trace=False, **trace_kwargs):
    from concourse.bass_utils import run_bass_kernel_spmd

    nc = _get_program()

    A = np.ascontiguousarray(A, dtype=np.float32)
    W = np.ascontiguousarray(W, dtype=np.float32)
    X = np.asarray(X, dtype=np.float32)
    Y = np.asarray(Y, dtype=np.float32)
    xt = np.ascontiguousarray(
        X.transpose(2, 0, 1).reshape(D, B * N), dtype=np.float16
    )  # [d, b*n]
    # [e_pair, par, b, m]: e = 2*e_pair + par
    yt = np.ascontiguousarray(
        Y.transpose(2, 0, 1).reshape(P, 2, B, N), dtype=np.float16
    )
    w_rep = np.ascontiguousarray(
        np.broadcast_to(W.reshape(1, KD), (P, KD)), dtype=np.float16
    )

    core_ids = list(range(C))
    in_maps = [
        {
            "a_sh": A[c * DL : (c + 1) * DL],
            "xt_in": np.ascontiguousarray(xt[c * DL : (c + 1) * DL]),
            "yt_in": yt,
            "w_rep": w_rep,
        }
        for c in core_ids
    ]

    res = run_bass_kernel_spmd(nc, in_maps, core_ids, trace=trace, **trace_kwargs)
    # each core returns its ReduceScatter n-slice [16, B*N]; concat + relayout
    s_nbm = np.concatenate(
        [np.asarray(res.results[c]["out"], dtype=np.float32) for c in core_ids],
        axis=0,
    ).reshape(N, B, N)
    out = np.ascontiguousarray(s_nbm.transpose(1, 0, 2))
    out = out + np.float32(b.reshape(-1)[0])
    return out, res


def kernel(X, Y, A, W, b):
    out, _ = _run(X, Y, A, W, b, trace=False)
    return out
